# revision 2
# baseline (speedup 1.0000x reference)
"""Bloom transformer block on 8 Trainium2 NeuronCores — v2.

Comm-free sharding as v1: core c handles batch c//4 and 512 of its 2048
tokens (two causally-nested 256-token q-tiles {r, r+4}, r = c%4); every
core redundantly computes LN1 + K/V projections for its full batch.

v2 restructures for a gapless TensorE stream (TRN2 p-states need ~3us of
continuous execution to reach the 2.4 GHz clock; every idle gap costs a
re-ramp):
 - LN1 h^T tiles for all 4 panels stay resident in SBUF; Q projection
   runs first (only needs panel 0, i.e. the core's own tokens) so the
   PE starts ~20us in, while LN of later panels proceeds on Vector.
 - K/V projections stream their weights (Activation-queue DMA) and are
   interleaved INTO the attention head loop, so the PE always has
   independent 512-free matmuls during attention's dependency bubbles.
 - ALiBi via per-partition Exp bias: softmax probs are invariant to
   per-query factors, so exp(scores + slope*(k - C_tile)) with
   C = q0+128 needs only an acol column as activation bias (no vector
   op); binary masks multiply the two diagonal tiles only.  Head 0
   (slope .707) would overflow exp over a 256-wide q tile and keeps the
   v1 scalar_tensor_tensor path.
 - x2 stays in SBUF across attention->MLP (no x2 DRAM round trip); LN2
   statistics are computed per o_proj column chunk so only the
   aggregation remains after the last chunk; h2^T transposes and MLP
   weight streams ride both hardware DMA queues (SP + Activation).

Exact-math shortcuts from v1 kept: k-bias dropped (cancels in softmax),
v-bias folded into the residual, LN affines folded into weights,
1/sqrt(HD) folded into Wq.
"""

import math
import os

import numpy as np
import ml_dtypes

import concourse.bass as bass
import concourse.tile as tile
from concourse import mybir
from concourse.bass_utils import run_bass_kernel_spmd

B, S, D, H = 2, 2048, 2048, 16
HD = D // H          # 128
FF = 4 * D           # 8192
EPS = 1e-5
NCORES = 8
GS = 4               # cores per batch (group size)
QT = S // GS         # own tokens per core = 512
NEG = -1.0e9
QW = 256             # q-tile width (tokens) in attention
NSLOT = QT // QW     # 2 slots per core
KEXT = [8, 16]       # padded k-extent (128-tiles) per slot
SK = sum(KEXT)
NKT = S // 128       # 16 k-tiles
DT16 = D // 128      # 16 feature tiles of 128
FT64 = FF // 128     # 64 FF tiles
QSCALE = 1.0 / math.sqrt(HD)

f32 = mybir.dt.float32
bf16 = mybir.dt.bfloat16


def _alibi_slopes(num_heads):
    closest = 2 ** math.floor(math.log2(num_heads))
    base = 2.0 ** (-(2.0 ** (-(math.log2(closest) - 3))))
    powers = np.arange(1, 1 + closest, dtype=np.float64)
    slopes = base ** powers
    if closest != num_heads:
        extra_base = 2.0 ** (-(2.0 ** (-(math.log2(2 * closest) - 3))))
        num_rem = min(closest, num_heads - closest)
        extra_powers = np.arange(1, 1 + 2 * num_rem, 2, dtype=np.float64)
        slopes = np.concatenate([slopes, extra_base ** extra_powers])
    return slopes.astype(np.float32)


# ---------------------------------------------------------------------------
# wait-split post-pass (single sync-wait per instruction on this walrus)
# ---------------------------------------------------------------------------
_ctr = [0]


def _split_waits(nc, maxw=1):
    for f in nc.m.functions:
        for bb in f.blocks:
            out = []
            changed = False
            for ins in bb.instructions:
                si = ins.sync_info
                waits = list(si.on_wait) if (si and si.on_wait) else []
                if len(waits) > maxw:
                    head, keep = waits[:-maxw], waits[-maxw:]
                    for w in head:
                        _ctr[0] += 1
                        nop = mybir.InstNoOp(name=f"I-waitsplit-{_ctr[0]}")
                        nop.engine = ins.engine
                        nop.sync_info = mybir.SyncInfo(on_wait=[w], on_update=[])
                        out.append(nop)
                    si.on_wait = keep
                    changed = True
                out.append(ins)
            if changed:
                bb.instructions = out
    return nc


# ---------------------------------------------------------------------------
# per-rank host-side structure
# ---------------------------------------------------------------------------
def _rank_structure(r):
    own256 = [r, r + 4]
    own128 = []
    for t in own256:
        own128 += [2 * t, 2 * t + 1]
    others128 = [t for t in range(NKT) if t not in own128]
    perm128 = own128 + others128
    klists = []
    for j in range(NSLOT):
        nown = 2 * j + 2
        kl = own128[:nown] + others128[: KEXT[j] - nown]
        klists.append(kl)
    return own256, perm128, klists


def _build_acol(r, slopes):
    """[128, H*SK] f32: slope_h * (k_orig - i*256); NEG on pad tiles."""
    _, _, klists = _rank_structure(r)
    out = np.empty((128, H * SK), dtype=np.float32)
    kk = np.arange(128, dtype=np.float64)
    for h in range(H):
        for j in range(NSLOT):
            i = r + 4 * j
            off = h * SK + sum(KEXT[:j])
            c = i * QW
            for p, kt in enumerate(klists[j]):
                if kt >= 2 * i + 2:          # fully beyond causal (pad)
                    out[:, off + p] = NEG
                else:
                    out[:, off + p] = slopes[h] * (kt * 128 + kk - c)
    return out


def _build_arow(slopes):
    """Shared [H, 3, 128, QW] f32 row tiles (subtracted from scores):
    var 0 = plain slope_h*qq; var 1/2 = plain + 1e9 on the causally-masked
    cells of the two own-diagonal k-tiles."""
    out = np.empty((H, 3, 128, QW), dtype=np.float32)
    kk = np.arange(128)
    qq = np.arange(QW)
    m0 = (kk[:, None] > qq[None, :]).astype(np.float32) * (-NEG)
    m1 = ((kk[:, None] + 128) > qq[None, :]).astype(np.float32) * (-NEG)
    for h in range(H):
        plain = np.broadcast_to(slopes[h] * qq[None, :].astype(np.float32),
                                (128, QW))
        out[h, 0] = plain
        out[h, 1] = plain + m0
        out[h, 2] = plain + m1
    return out


# ---------------------------------------------------------------------------
# device program (identical for all cores)
# ---------------------------------------------------------------------------
def build_nc(debug=False):
    nc = bass.Bass(target_bir_lowering=False)

    xp = nc.dram_tensor("xp", [S, D], f32, kind="ExternalInput")
    xres = nc.dram_tensor("xres", [QT, D], f32, kind="ExternalInput")
    wqkv = nc.dram_tensor("wqkv", [D, 3 * D], bf16, kind="ExternalInput")
    bq_pp = nc.dram_tensor("bq_pp", [128, DT16], f32, kind="ExternalInput")
    wo = nc.dram_tensor("wo", [D, D], bf16, kind="ExternalInput")
    w1 = nc.dram_tensor("w1", [D, FF], bf16, kind="ExternalInput")
    b1_pp = nc.dram_tensor("b1_pp", [128, FT64], f32, kind="ExternalInput")
    w2 = nc.dram_tensor("w2", [FF, D], bf16, kind="ExternalInput")
    b2_bc = nc.dram_tensor("b2_bc", [128, D], f32, kind="ExternalInput")
    acol = nc.dram_tensor("acol", [128, H * SK], f32, kind="ExternalInput")
    arow = nc.dram_tensor("arow", [H, 3, 128, QW], f32,
                          kind="ExternalInput")

    out = nc.dram_tensor("out", [QT, D], f32, kind="ExternalOutput")

    ikind = "ExternalOutput" if debug else "Internal"
    h_dram = nc.dram_tensor("h_dram", [S, D], bf16, kind=ikind)
    kT_dram = nc.dram_tensor("kT_dram", [D, S], bf16, kind=ikind)
    v_dram = nc.dram_tensor("v_dram", [S, D], bf16, kind=ikind)
    h2_dram = nc.dram_tensor("h2_dram", [QT, D], bf16, kind=ikind)
    rec_dram = nc.dram_tensor("rec_dram", [NSLOT * H, QW], f32, kind="Internal")

    with tile.TileContext(nc) as tc:
        with tc.tile_pool(name="persist", bufs=1) as pp:
            ones = pp.tile([128, 1], bf16, tag="ones")
            nc.vector.memset(ones, 1.0)
            eps_t = pp.tile([128, 1], f32, tag="eps")
            nc.vector.memset(eps_t, EPS)
            b1_sb = pp.tile([128, FT64], f32, tag="b1")
            nc.scalar.dma_start(out=b1_sb, in_=b1_pp.ap())
            acol_sb = pp.tile([128, H * SK], f32, tag="acol")
            nc.scalar.dma_start(out=acol_sb, in_=acol.ap())
            _body(nc, tc, xp, xres, wqkv, bq_pp, wo, w1, w2, b2_bc,
                  h_dram, kT_dram, v_dram, h2_dram, rec_dram, out, arow,
                  ones, eps_t, b1_sb, acol_sb)

    _split_waits(nc)
    return nc


def _layernorm_tile(nc, pool, xt, eps_t, out_dtype=bf16):
    """token-major LN on a [128, D] f32 tile (w/b folded into weights)."""
    stats = pool.tile([128, 4, 6], f32, tag="lnstats")
    xg = xt.rearrange("p (n f) -> p n f", f=512)
    for i in range(4):
        nc.vector.bn_stats(out=stats[:, i, :], in_=xg[:, i, :])
    mv = pool.tile([128, 2], f32, tag="lnmv")
    nc.vector.bn_aggr(out=mv, in_=stats)
    rs = pool.tile([128, 1], f32, tag="lnrs")
    nc.scalar.activation(out=rs, in_=mv[:, 1:2],
                         func=mybir.ActivationFunctionType.Sqrt,
                         bias=eps_t, scale=1.0)
    nc.vector.reciprocal(out=rs, in_=rs)
    h = pool.tile([128, D], out_dtype, tag="lnh")
    nc.vector.tensor_scalar(out=h, in0=xt, scalar1=mv[:, 0:1], scalar2=rs,
                            op0=mybir.AluOpType.subtract,
                            op1=mybir.AluOpType.mult)
    return h


def _body(nc, tc, xp, xres, wqkv, bq_pp, wo, w1, w2, b2_bc,
          h_dram, kT_dram, v_dram, h2_dram, rec_dram, out, arow,
          ones, eps_t, b1_sb, acol_sb):
    # -------------------------------------------------------------------
    # Phase A: LN1 -> h^T resident; Q proj; K/V projection streamers
    # -------------------------------------------------------------------
    pA = tc.alloc_tile_pool(name="pA", bufs=1, side="left")
    sbA = tc.alloc_tile_pool(name="pAx", bufs=2, side="left")
    wkq = tc.alloc_tile_pool(name="pAwk", bufs=2, side="left")
    wvp = tc.alloc_tile_pool(name="pAwv", bufs=1, side="left")
    psKV = tc.alloc_tile_pool(name="psKV", bufs=2, side="left", space="PSUM")

    bq_sb = pA.tile([128, DT16], f32, tag="bq")
    nc.scalar.dma_start(out=bq_sb, in_=bq_pp.ap())

    hT = [[pA.tile([128, 512], bf16, tag=f"hT{pan}_{dt}",
                   name=f"hT{pan}_{dt}") for dt in range(DT16)]
          for pan in range(4)]

    for pan in range(4):
        for t in range(4):
            row0 = pan * 512 + t * 128
            xt = sbA.tile([128, D], f32, tag="x")
            nc.sync.dma_start(out=xt, in_=xp.ap()[row0:row0 + 128, :])
            h = _layernorm_tile(nc, sbA, xt, eps_t)
            nc.sync.dma_start(out=h_dram.ap()[row0:row0 + 128, :], in_=h)
        for dt in range(DT16):
            nc.sync.dma_start_transpose(
                out=hT[pan][dt],
                in_=h_dram.ap()[pan * 512:(pan + 1) * 512,
                                dt * 128:(dt + 1) * 128])

    # qT / attnT live until phase D start; right-side pool below kvpool
    qTp = tc.alloc_tile_pool(name="pQT", bufs=1, side="right")
    qT = [qTp.tile([128, QT], bf16, tag=f"qT{m}", name=f"qT{m}")
          for m in range(DT16)]
    attnT = {}
    for h in range(H):
        for j in range(NSLOT):
            attnT[(h, j)] = qTp.tile([128, QW], bf16, tag=f"at{h}_{j}",
                                     name=f"at{h}_{j}")

    # Q projection (panel 0 = own 512 tokens) — first tensor work
    psQ = tc.alloc_tile_pool(name="psQ", bufs=2, side="right", space="PSUM")
    for m in range(DT16):
        wq = wkq.tile([128, DT16, 128], bf16, tag="wq")
        nc.scalar.dma_start(
            out=wq,
            in_=wqkv.ap()[:, m * 128:(m + 1) * 128]
            .rearrange("(dt p) f -> p dt f", p=128))
        ps = psQ.tile([128, QT], f32)
        for dt in range(DT16):
            nc.tensor.matmul(ps, wq[:, dt, :], hT[0][dt],
                             start=(dt == 0), stop=(dt == DT16 - 1))
        nc.vector.tensor_scalar(out=qT[m], in0=ps,
                                scalar1=bq_sb[:, m:m + 1],
                                scalar2=None,
                                op0=mybir.AluOpType.add)
    psQ.release()

    def k_proj(m):
        wkt = wkq.tile([128, DT16, 128], bf16, tag="wk")
        nc.scalar.dma_start(
            out=wkt,
            in_=wqkv.ap()[:, D + m * 128:D + (m + 1) * 128]
            .rearrange("(dt p) f -> p dt f", p=128))
        for pan in range(4):
            ps = psKV.tile([128, 512], f32, tag="pskv", name="psk")
            for dt in range(DT16):
                nc.tensor.matmul(ps, wkt[:, dt, :], hT[pan][dt],
                                 start=(dt == 0), stop=(dt == DT16 - 1))
            kt = sbA.tile([128, 512], bf16, tag="kout")
            nc.scalar.copy(out=kt, in_=ps)
            nc.sync.dma_start(
                out=kT_dram.ap()[m * 128:(m + 1) * 128,
                                 pan * 512:(pan + 1) * 512], in_=kt)

    def v_proj(nch):
        col0 = 2 * D + nch * 512
        wv_lo = wvp.tile([128, 8, 512], bf16, tag="wvl")
        nc.scalar.dma_start(
            out=wv_lo,
            in_=wqkv.ap()[0:1024, col0:col0 + 512]
            .rearrange("(dt p) f -> p dt f", p=128))
        wv_hi = wvp.tile([128, 8, 512], bf16, tag="wvh")
        nc.scalar.dma_start(
            out=wv_hi,
            in_=wqkv.ap()[1024:2048, col0:col0 + 512]
            .rearrange("(dt p) f -> p dt f", p=128))
        for t in range(16):
            ps = psKV.tile([128, 512], f32, tag="pskv", name="psv")
            for dt in range(DT16):
                wv = wv_lo if dt < 8 else wv_hi
                nc.tensor.matmul(
                    ps, hT[t // 4][dt][:, (t % 4) * 128:(t % 4 + 1) * 128],
                    wv[:, dt % 8, :],
                    start=(dt == 0), stop=(dt == DT16 - 1))
            vt = sbA.tile([128, 512], bf16, tag="vout")
            nc.scalar.copy(out=vt, in_=ps)
            nc.sync.dma_start(
                out=v_dram.ap()[t * 128:(t + 1) * 128,
                                nch * 512:(nch + 1) * 512], in_=vt)

    for m in range(6):
        k_proj(m)
    v_proj(0)

    # -------------------------------------------------------------------
    # Phase B: attention head loop with interleaved K/V production
    # -------------------------------------------------------------------
    kvpool = tc.alloc_tile_pool(name="pBkv", bufs=2, side="right")
    aqpool = tc.alloc_tile_pool(name="pBaq", bufs=2, side="right")
    spool = tc.alloc_tile_pool(name="pBs", bufs=4, side="right")
    denpool = tc.alloc_tile_pool(name="pBden", bufs=4, side="right")
    psS = tc.alloc_tile_pool(name="psS", bufs=2, side="right", space="PSUM")
    psAV = tc.alloc_tile_pool(name="psAV", bufs=2, side="right", space="PSUM")
    psD = tc.alloc_tile_pool(name="psD", bufs=1, side="right", space="PSUM")

    # K-proj emission schedule: all done by end of head 6
    ksched = {0: [6], 1: [7], 2: [8], 3: [9], 4: [10],
              5: [11, 12], 6: [13, 14, 15]}
    vsched = {1: 1, 3: 2, 5: 3}

    wopool = None
    wo_sb = []

    for h in range(H):
        kt_sb = kvpool.tile([128, NKT * 128], bf16, tag="kt")
        nc.sync.dma_start(out=kt_sb,
                          in_=kT_dram.ap()[h * 128:(h + 1) * 128, :])
        v_sb = kvpool.tile([128, NKT, 128], bf16, tag="vt")
        nc.sync.dma_start(
            out=v_sb,
            in_=v_dram.ap()[:, h * 128:(h + 1) * 128]
            .rearrange("(n p) f -> p n f", p=128))
        ar_sb = aqpool.tile([128, 3, QW], f32, tag="ar")
        nc.scalar.dma_start(out=ar_sb,
                            in_=arow.ap()[h].rearrange("v p f -> p v f"))

        for j in range(NSLOT):
            ext = KEXT[j]
            nown = 2 * j + 2
            pav = psAV.tile([128, QW], f32)
            pden = psD.tile([1, QW], f32)
            qslice = qT[h][:, j * QW:(j + 1) * QW]
            for p in range(ext):
                permpos = p if p < nown else p + (4 - nown)
                var = 1 if p == 2 * j else (2 if p == 2 * j + 1 else 0)
                cidx = h * SK + sum(KEXT[:j]) + p
                ps = psS.tile([128, QW], f32)
                nc.tensor.matmul(
                    ps, kt_sb[:, permpos * 128:(permpos + 1) * 128],
                    qslice, start=True, stop=True)
                ss = spool.tile([128, QW], f32, tag="ss")
                nc.vector.scalar_tensor_tensor(
                    out=ss, in0=ps, scalar=acol_sb[:, cidx:cidx + 1],
                    in1=ar_sb[:, var, :],
                    op0=mybir.AluOpType.add,
                    op1=mybir.AluOpType.subtract)
                es = spool.tile([128, QW], bf16, tag="es")
                nc.scalar.activation(
                    out=es, in_=ss,
                    func=mybir.ActivationFunctionType.Exp)
                nc.tensor.matmul(pden, ones, es,
                                 start=(p == 0), stop=(p == ext - 1))
                nc.tensor.matmul(pav, v_sb[:, permpos, :], es,
                                 start=(p == 0), stop=(p == ext - 1))
            rec = denpool.tile([1, QW], f32, tag="rec")
            nc.vector.reciprocal(out=rec, in_=pden)
            row = rec_dram.ap()[j * H + h:j * H + h + 1, :]
            nc.sync.dma_start(out=row, in_=rec)
            recb = denpool.tile([128, QW], f32, tag="recb")
            bc = bass.AP(tensor=row.tensor, offset=row.offset,
                         ap=[[0, 128]] + list(row.ap[1:]))
            nc.gpsimd.dma_start(out=recb, in_=bc)
            nc.vector.tensor_mul(out=attnT[(h, j)], in0=pav, in1=recb)

        for mm in ksched.get(h, []):
            k_proj(mm)
        if h in vsched:
            v_proj(vsched[h])
        if h == 7:
            # all K/V emitted; free phase-A SBUF/PSUM, stream wo into it
            psKV.release()
            wvp.release()
            wkq.release()
            sbA.release()
            pA.release()
            wopool = tc.alloc_tile_pool(name="pCwo", bufs=1, side="right")
            for ht in range(DT16):
                wot = wopool.tile([128, D], bf16, tag=f"wo{ht}")
                nc.scalar.dma_start(out=wot,
                                    in_=wo.ap()[ht * 128:(ht + 1) * 128, :])
                wo_sb.append(wot)

    psD.release()
    psAV.release()
    psS.release()

    # -------------------------------------------------------------------
    # Phase C: o_proj + residual -> x2 (SBUF); LN2 stats inline; h2^T
    # -------------------------------------------------------------------
    x2p = tc.alloc_tile_pool(name="pCx2", bufs=1, side="left")
    h2pool = tc.alloc_tile_pool(name="pCh2", bufs=1, side="left")
    opool = tc.alloc_tile_pool(name="pCo", bufs=3, side="left")
    psO = tc.alloc_tile_pool(name="psO", bufs=2, side="left", space="PSUM")

    x2 = [x2p.tile([128, D], f32, tag=f"x2_{t}", name=f"x2_{t}")
          for t in range(4)]
    st2 = [x2p.tile([128, 4, 6], f32, tag=f"st2_{t}", name=f"st2_{t}")
           for t in range(4)]
    h2T = [h2pool.tile([128, QT], bf16, tag=f"h2T{dt}", name=f"h2T{dt}")
           for dt in range(DT16)]

    for t in range(4):
        j, tt = t // 2, t % 2
        for dc in range(4):
            ps = psO.tile([128, 512], f32)
            for h in range(H):
                nc.tensor.matmul(
                    ps, attnT[(h, j)][:, tt * 128:(tt + 1) * 128],
                    wo_sb[h][:, dc * 512:(dc + 1) * 512],
                    start=(h == 0), stop=(h == H - 1))
            xr = opool.tile([128, 512], f32, tag="xr")
            nc.sync.dma_start(
                out=xr, in_=xres.ap()[t * 128:(t + 1) * 128,
                                      dc * 512:(dc + 1) * 512])
            xc = x2[t][:, dc * 512:(dc + 1) * 512]
            nc.vector.tensor_add(out=xc, in0=ps, in1=xr)
            nc.vector.bn_stats(out=st2[t][:, dc, :], in_=xc)
        # LN2 finalize for tile t
        mv = opool.tile([128, 2], f32, tag="lnmv")
        nc.vector.bn_aggr(out=mv, in_=st2[t])
        rs = opool.tile([128, 1], f32, tag="lnrs")
        nc.scalar.activation(out=rs, in_=mv[:, 1:2],
                             func=mybir.ActivationFunctionType.Sqrt,
                             bias=eps_t, scale=1.0)
        nc.vector.reciprocal(out=rs, in_=rs)
        h2 = opool.tile([128, D], bf16, tag="lnh")
        nc.vector.tensor_scalar(out=h2, in0=x2[t], scalar1=mv[:, 0:1],
                                scalar2=rs,
                                op0=mybir.AluOpType.subtract,
                                op1=mybir.AluOpType.mult)
        nc.sync.dma_start(out=h2_dram.ap()[t * 128:(t + 1) * 128, :],
                          in_=h2)
    for dt in range(DT16):
        nc.sync.dma_start_transpose(
            out=h2T[dt], in_=h2_dram.ap()[:, dt * 128:(dt + 1) * 128])

    # -------------------------------------------------------------------
    # Phase D: GELU MLP + residual
    # -------------------------------------------------------------------
    psO.release()
    opool.release()
    wopool.release()
    denpool.release()
    spool.release()
    aqpool.release()
    kvpool.release()
    qTp.release()

    mpool = tc.alloc_tile_pool(name="pDm", bufs=1, side="right")
    w1pool = tc.alloc_tile_pool(name="pDw1", bufs=2, side="right")
    w2pool = tc.alloc_tile_pool(name="pDw2", bufs=2, side="right")
    x2pool = tc.alloc_tile_pool(name="pDx2", bufs=2, side="right")
    psM1 = tc.alloc_tile_pool(name="psM1", bufs=3, side="left", space="PSUM")
    psM2 = tc.alloc_tile_pool(name="psM2", bufs=4, side="left", space="PSUM")

    b2_sb = mpool.tile([128, D], f32, tag="b2")
    nc.scalar.dma_start(out=b2_sb, in_=b2_bc.ap())

    m1 = []
    for m in range(FT64):
        w1t = w1pool.tile([128, DT16, 128], bf16, tag="w1")
        nc.scalar.dma_start(
            out=w1t,
            in_=w1.ap()[:, m * 128:(m + 1) * 128]
            .rearrange("(dt p) f -> p dt f", p=128))
        ps = psM1.tile([128, QT], f32)
        for dt in range(DT16):
            nc.tensor.matmul(ps, w1t[:, dt, :], h2T[dt],
                             start=(dt == 0), stop=(dt == DT16 - 1))
        mt = mpool.tile([128, QT], bf16, tag=f"m1_{m}")
        nc.scalar.activation(
            out=mt, in_=ps,
            func=mybir.ActivationFunctionType.Gelu_apprx_tanh,
            bias=b1_sb[:, m:m + 1], scale=1.0)
        m1.append(mt)

    NQ = 16
    for dc in range(4):
        pss = [psM2.tile([128, 512], f32, name=f"psm2_{t}", tag="psm2")
               for t in range(4)]
        for qc in range(4):
            w2t = w2pool.tile([128, NQ, 512], bf16, tag="w2")
            nc.scalar.dma_start(
                out=w2t,
                in_=w2.ap()[qc * NQ * 128:(qc + 1) * NQ * 128,
                            dc * 512:(dc + 1) * 512]
                .rearrange("(ft p) f -> p ft f", p=128))
            for t in range(4):
                for f in range(NQ):
                    ft = qc * NQ + f
                    nc.tensor.matmul(
                        pss[t], m1[ft][:, t * 128:(t + 1) * 128],
                        w2t[:, f, :],
                        start=(ft == 0), stop=(ft == FT64 - 1))
        for t in range(4):
            s1 = x2pool.tile([128, 512], f32, tag="s1")
            nc.vector.tensor_add(out=s1, in0=pss[t],
                                 in1=x2[t][:, dc * 512:(dc + 1) * 512])
            o = x2pool.tile([128, 512], f32, tag="o")
            nc.vector.tensor_add(out=o, in0=s1,
                                 in1=b2_sb[:, dc * 512:(dc + 1) * 512])
            nc.sync.dma_start(
                out=out.ap()[t * 128:(t + 1) * 128,
                             dc * 512:(dc + 1) * 512], in_=o)

    psM2.release()
    psM1.release()
    x2pool.release()
    w2pool.release()
    w1pool.release()
    mpool.release()
    h2pool.release()
    x2p.release()


# ---------------------------------------------------------------------------
# host wrapper
# ---------------------------------------------------------------------------
_nc_cache = {}


def _get_nc(debug=False):
    if debug not in _nc_cache:
        _nc_cache[debug] = build_nc(debug=debug)
    return _nc_cache[debug]


def _prep_inputs(x, ln1_w, ln1_b, wqkv, bqkv, wo, bo, ln2_w, ln2_b,
                 w1, b1, w2, b2):
    slopes = _alibi_slopes(H)
    wqkv_f = (ln1_w[:, None] * wqkv).astype(np.float32)
    bqkv_f = (ln1_b @ wqkv + bqkv).astype(np.float32)
    wqkv_f[:, :D] *= QSCALE
    bqkv_f[:D] *= QSCALE
    w1_f = (ln2_w[:, None] * w1).astype(np.float32)
    b1_f = (ln2_b @ w1 + b1).astype(np.float32)

    wqkv_b = wqkv_f.astype(ml_dtypes.bfloat16)
    wo_b = wo.astype(ml_dtypes.bfloat16)
    w1_b = w1_f.astype(ml_dtypes.bfloat16)
    w2_b = w2.astype(ml_dtypes.bfloat16)

    bq_pp = bqkv_f[:D].reshape(DT16, 128).T.copy().astype(np.float32)
    b1_pp = b1_f.reshape(FT64, 128).T.copy().astype(np.float32)
    b2_bc = np.broadcast_to(b2.astype(np.float32), (128, D)).copy()
    res_const = (bo + bqkv_f[2 * D:] @ wo).astype(np.float32)
    arow = _build_arow(slopes)

    in_maps = []
    metas = []
    for c in range(NCORES):
        batch, r = divmod(c, GS)
        _, perm128, _ = _rank_structure(r)
        perm_tok = np.concatenate(
            [np.arange(t * 128, (t + 1) * 128) for t in perm128])
        xp = np.ascontiguousarray(x[batch][perm_tok]).astype(np.float32)
        xr = (xp[:QT] + res_const[None, :]).astype(np.float32)
        in_maps.append({
            "xp": xp, "xres": xr,
            "wqkv": wqkv_b, "bq_pp": bq_pp,
            "wo": wo_b, "w1": w1_b, "b1_pp": b1_pp,
            "w2": w2_b, "b2_bc": b2_bc,
            "acol": _build_acol(r, slopes), "arow": arow,
        })
        metas.append((batch, perm_tok[:QT]))
    return in_maps, metas


last_result = None


def _install_ntff_hook_shim():
    import sys as _sys
    import types
    if "antenv.axon_hooks" not in _sys.modules:
        import importlib
        tb = importlib.import_module("trn_agent_boot.trn_boot")
        hook = tb._ntff_profile_via_ctypes("/opt/axon/libaxon_pjrt.so")
        mod = types.ModuleType("antenv.axon_hooks")
        mod.get_axon_ntff_profile_hook = lambda: hook
        _sys.modules["antenv.axon_hooks"] = mod
    import concourse.bass_utils as bu
    bu.upload_artifacts = lambda tmpdir: "(upload disabled)"


def kernel(**inputs):
    global last_result
    args = {k: np.asarray(v, dtype=np.float32) for k, v in inputs.items()}
    in_maps, metas = _prep_inputs(
        args["x"], args["ln1_w"], args["ln1_b"], args["wqkv"], args["bqkv"],
        args["wo"], args["bo"], args["ln2_w"], args["ln2_b"],
        args["w1"], args["b1"], args["w2"], args["b2"])
    nc = _get_nc()
    kwargs = {}
    if os.environ.get("KBENCH_TRACE"):
        _install_ntff_hook_shim()
        kwargs = dict(trace=True, trace_cores=list(range(NCORES)))
    res = run_bass_kernel_spmd(nc, in_maps, core_ids=list(range(NCORES)),
                               **kwargs)
    last_result = res
    out = np.empty((B, S, D), dtype=np.float32)
    for c in range(NCORES):
        batch, tok = metas[c]
        out[batch, tok] = res.results[c]["out"]
    return out
